# revision 2
# baseline (speedup 1.0000x reference)
"""Trainium2 Bass kernel for the DCE 2CXM signal model — exp-Muentz basis.

Math: conc[k,n] = c1[n]*U[k,n] + c2[n]*V[k,n] with U[k,n] = F_k(theta_m[n]),
V[k,n] = F_k(theta_p[n]), where F_k(th) = sum_t A[k,t] exp(-0.1*t*th) is
approximated by a 64-term exponential sum F_k(th) ~= sum_j C[k,j]
exp(-0.1*s_j*th) fitted on the data's theta range (host, float64 ridge).
The c1/c2 normalization scalars collapse the he/hp normalizations
(geometric closed forms), as in the previous kernel.

The SPGR epilogue is rewritten via 1/(1-e^v) = 1/2 - coth(v/2)/2 so the
whole main loop needs only Exp + Tanh — both live in the same activation
table set (exp_and_others): zero table switches in the loop.

Device layout per core (12800 pixels):
 - prep in pixel-major [128, 100] (per-element ops cost free-size only),
 - rows [50, 512]: partition 2j = theta_m / c1 of subtile j, 2j+1 = theta_p / c2,
 - per 2-subtile iteration: PE K=2 ones-matmul broadcasts theta rows into
   PSUM [128, 1024] (rows 0-63 theta_m, 64-127 theta_p); ACT computes the
   exp basis with per-partition scale -0.1*s_j; PE broadcasts c1/c2 the same
   way; DVE+Pool scale the basis; PE contracts (K=64, accumulating the c1-
   and c2-scaled halves) into conc PSUM [100, 512]; ACT tanh; DVE recip;
   Pool affine; DMA out.
"""

import os
from contextlib import ExitStack

import numpy as np

H = W = 320
NPIX = H * W
NCORES = 8
SHARD = NPIX // NCORES      # 12800
NT = 512                    # pixels per subtile
NTILES = SHARD // NT        # 25
PC = 100                    # prep cols: [128, 100] pixel-major
TS = 50
KB = 64                     # basis size
STEP = 0.1
DELAY = 30
L = 589

SIG_BASELINE = 100.0
R1 = 1.0
R1CA = 4.3
FA = 10.0
TR = 0.00487

_CACHE: dict = {}


def _spgr_consts():
    f32 = np.float32
    fa = FA * np.pi / 180.0
    cosf = float(np.cos(f32(fa)))
    sinf = float(np.sin(f32(fa)))
    E1 = float(np.exp(f32(-TR * R1)))
    M0 = SIG_BASELINE * (1.0 - cosf * E1) / (sinf * (1.0 - E1))
    M0t = M0 * sinf
    M_st = M0t * (1.0 - E1) / (1.0 - E1 * cosf)
    C0 = SIG_BASELINE - M_st
    K1 = C0 + M0t / cosf
    K2 = M0t * (cosf - 1.0) / cosf
    K1p = K1 + K2 / 2.0
    K2p = -K2 / 2.0
    VH0 = 0.5 * (-TR * R1 + np.log(cosf))
    return K1p, K2p, VH0


def _build_bass():
    import concourse.bass as bass
    import concourse.tile as tile
    from concourse import bacc, mybir

    f32 = mybir.dt.float32
    f32r = mybir.dt.float32r
    AF = mybir.ActivationFunctionType
    ALU = mybir.AluOpType

    K1p, K2p, VH0 = _spgr_consts()

    nc = bacc.Bacc()
    pmap = nc.dram_tensor("pmap", [4, SHARD], f32, kind="ExternalInput")
    cmat = nc.dram_tensor("cmat", [KB, TS], f32, kind="ExternalInput")
    oblk = nc.dram_tensor("oblk", [2, 128], f32, kind="ExternalInput")
    svec = nc.dram_tensor("svec", [128, 1], f32, kind="ExternalInput")
    sig = nc.dram_tensor("sig", [TS, SHARD], f32, kind="ExternalOutput")

    with tile.TileContext(nc) as tc, ExitStack() as ctx:
        const = ctx.enter_context(tc.tile_pool(name="const", bufs=1))
        thps = ctx.enter_context(
            tc.tile_pool(name="thps", bufs=2, space=bass.MemorySpace.PSUM))
        cps = ctx.enter_context(
            tc.tile_pool(name="cps", bufs=1, space=bass.MemorySpace.PSUM))
        ccps = ctx.enter_context(
            tc.tile_pool(name="ccps", bufs=1, space=bass.MemorySpace.PSUM))
        baspool = ctx.enter_context(tc.tile_pool(name="bas", bufs=2))
        bspool = ctx.enter_context(tc.tile_pool(name="bs", bufs=2))
        epool = ctx.enter_context(tc.tile_pool(name="ep", bufs=2))
        opool = ctx.enter_context(tc.tile_pool(name="op", bufs=2))
        rows = ctx.enter_context(tc.tile_pool(name="rows", bufs=1))
        prep = ctx.enter_context(tc.tile_pool(name="prep", bufs=1))

        V = nc.vector
        G = nc.gpsimd

        # cmat duplicated at partitions 0:64 and 64:128 so lhsT base matches
        # whichever half of the basis the rhs slice starts at.
        cmat_sb = const.tile([128, TS], f32, tag="cmat_sb", name="cmat_sb")
        cmat_r = const.tile([128, TS], f32r, tag="cmat_r", name="cmat_r")
        oblk_sb = const.tile([2, 128], f32, tag="oblk_sb", name="oblk_sb")
        oblk_r = const.tile([2, 128], f32r, tag="oblk_r", name="oblk_r")
        sv_sb = const.tile([128, 1], f32, tag="sv_sb", name="sv_sb")
        b_vh = const.tile([128, 1], f32, tag="b_vh", name="b_vh")
        nc.sync.dma_start(out=cmat_sb[0:KB, :], in_=cmat[:])
        nc.sync.dma_start(out=cmat_sb[KB:128, :], in_=cmat[:])
        nc.sync.dma_start(out=oblk_sb, in_=oblk[:])
        nc.sync.dma_start(out=sv_sb, in_=svec[:])
        V.tensor_copy(cmat_r, cmat_sb)
        V.tensor_copy(oblk_r, oblk_sb)
        V.memset(b_vh, float(VH0))

        # ---------------- prep: pixel-major [128, 100] ----------------
        def pt(tag):
            return prep.tile([128, PC], f32, tag=tag, name=tag)

        ve, vp, fp, ps = (pt(t) for t in ("ve", "vp", "fp", "ps"))
        for i, t in enumerate((ve, vp, fp, ps)):
            nc.sync.dma_start(
                out=t, in_=pmap[i, :].rearrange("(p c) -> p c", p=128))

        rfp = pt("rfp"); V.reciprocal_approx_fast(rfp, fp)
        rps = pt("rps"); V.reciprocal_approx_fast(rps, ps)
        Te = pt("Te"); V.tensor_mul(Te, ve, rps)
        s_ = pt("s_"); G.tensor_add(s_, vp, ve)
        T_ = pt("T_"); V.tensor_mul(T_, s_, rfp)          # (vp+ve)/fp
        Tc = pt("Tc"); G.tensor_mul(Tc, vp, rfp)
        V.tensor_add(s_, T_, Te)                           # s = T+Te
        m4 = pt("m4")
        V.scalar_tensor_tensor(m4, Tc, 4.0, Te, op0=ALU.mult, op1=ALU.mult)
        sq = pt("sq"); V.tensor_mul(sq, s_, s_)
        V.tensor_sub(sq, sq, m4)
        disc = T_
        nc.scalar.sqrt(disc, sq)                           # SQRT table
        den = pt("den"); V.tensor_add(den, s_, disc)
        rden = pt("rden"); V.reciprocal_approx_fast(rden, den)
        thm = pt("thm"); V.tensor_scalar_mul(thm, rden, 2.0)
        rm4 = pt("rm4"); V.reciprocal_approx_fast(rm4, m4)
        thp = pt("thp")
        V.scalar_tensor_tensor(thp, den, 2.0, rm4, op0=ALU.mult, op1=ALU.mult)

        # geometric sums Sm, Sp  (EXP table; loop stays on this table)
        def geo(theta, tag):
            r1 = pt(tag + "_r1")
            nc.scalar.activation(r1, theta, AF.Exp, bias=0.0, scale=-STEP)
            rl = pt(tag + "_rl")
            nc.scalar.activation(rl, theta, AF.Exp, bias=0.0, scale=-STEP * L)
            V.tensor_scalar(rl, rl, -1.0, 1.0, op0=ALU.mult, op1=ALU.add)
            V.tensor_scalar(r1, r1, -1.0, 1.0, op0=ALU.mult, op1=ALU.add)
            V.reciprocal_approx_fast(r1, r1)
            V.tensor_mul(rl, rl, r1)
            return rl

        Sm = geo(thm, "gm")
        Sp = geo(thp, "gp")

        alp = pt("alp"); G.tensor_mul(alp, Te, thm)
        G.tensor_scalar(alp, alp, -1.0, 1.0, op0=ALU.mult, op1=ALU.add)
        bet = pt("bet"); G.tensor_mul(bet, Te, thp)
        G.tensor_scalar_sub(bet, bet, 1.0)

        de = pt("de"); V.tensor_sub(de, Sm, Sp)
        V.reciprocal_approx_fast(de, de)
        V.tensor_mul(Sm, alp, Sm)
        V.tensor_mul(Sp, bet, Sp)
        V.tensor_add(Sm, Sm, Sp)
        V.reciprocal_approx_fast(Sm, Sm)                   # 1/(a*Sm+b*Sp)
        u_ = rden
        V.tensor_mul(u_, ve, de)                           # ve/(Sm-Sp)
        V.tensor_mul(alp, alp, Sm)
        c1 = pt("c1"); V.tensor_mul(c1, vp, alp)
        V.tensor_add(c1, c1, u_)
        V.tensor_mul(bet, bet, Sm)
        c2 = pt("c2"); V.tensor_mul(c2, vp, bet)
        V.tensor_sub(c2, c2, u_)

        # rows: [2, SHARD] f32r (partition 0 = m-quantity, 1 = p-quantity);
        # f32r rounding happens in cheap pixel-major [128, 100] copies first.
        def to_rows(src_m, src_p, tag):
            t = rows.tile([2, SHARD], f32r, tag=tag, name=tag)
            for row, src in ((0, src_m), (1, src_p)):
                rsrc = prep.tile([128, PC], f32r, tag=tag + f"_rr{row}",
                                 name=tag + f"_rr{row}")
                V.tensor_copy(rsrc, src)
                nc.sync.dma_start(out=t[row:row + 1, :], in_=rsrc)
            return t

        rows_th = to_rows(thm, thp, "r_th")
        rows_c = to_rows(c1, c2, "r_c")

        # ---------------- main loop ----------------
        groups = []
        j = 0
        while j < NTILES:
            b = min(2, NTILES - j)
            groups.append((j, b))
            j += b
        ng = int(os.environ.get("DCE_NGROUPS", "0"))
        if ng:
            groups = groups[:ng]
        if os.environ.get("DCE_PREPONLY"):
            groups = []
        if groups or True:
            pass

        for j0, b in groups:
            Wg = b * NT
            STAGE = int(os.environ.get("DCE_STAGE", "8"))
            th_ps = thps.tile([128, 1024], f32, tag="th_ps", name="th_ps")
            for h in range(b):
                nc.tensor.matmul(
                    th_ps[:, h * NT:(h + 1) * NT], oblk_r,
                    rows_th[0:2, (j0 + h) * NT:(j0 + h + 1) * NT],
                    start=True, stop=True)
            if STAGE < 2:
                dbg = opool.tile([TS, 1024], f32, tag="dbg", name="dbg")
                V.tensor_copy(dbg[:, :Wg], th_ps[0:TS, :Wg])
                nc.sync.dma_start(out=sig[:, j0 * NT:j0 * NT + Wg], in_=dbg[:, :Wg])
                continue
            bas = baspool.tile([128, 1024], f32r, tag="bas", name="bas")
            nc.scalar.activation(
                bas[:, :Wg], th_ps[:, :Wg], AF.Exp,
                bias=0.0, scale=sv_sb[:, 0:1])

            if STAGE < 3:
                dbg = opool.tile([TS, 1024], f32, tag="dbg", name="dbg")
                V.tensor_copy(dbg[:, :Wg], bas[0:TS, :Wg].bitcast(f32))
                nc.sync.dma_start(out=sig[:, j0 * NT:j0 * NT + Wg], in_=dbg[:, :Wg])
                continue
            c_ps = cps.tile([128, 1024], f32, tag="c_ps", name="c_ps")
            for h in range(b):
                nc.tensor.matmul(
                    c_ps[:, h * NT:(h + 1) * NT], oblk_r,
                    rows_c[0:2, (j0 + h) * NT:(j0 + h + 1) * NT],
                    start=True, stop=True)

            if STAGE < 4:
                dbg = opool.tile([TS, 1024], f32, tag="dbg", name="dbg")
                V.tensor_copy(dbg[:, :Wg], c_ps[0:TS, :Wg])
                nc.sync.dma_start(out=sig[:, j0 * NT:j0 * NT + Wg], in_=dbg[:, :Wg])
                continue
            bs = bspool.tile([128, 1024], f32r, tag="bs", name="bs")
            V.tensor_mul(bs[:, :Wg], bas[:, :Wg].bitcast(f32), c_ps[:, :Wg])

            if STAGE < 5:
                dbg = opool.tile([TS, 1024], f32, tag="dbg", name="dbg")
                V.tensor_copy(dbg[:, :Wg], bs[0:TS, :Wg].bitcast(f32))
                nc.sync.dma_start(out=sig[:, j0 * NT:j0 * NT + Wg], in_=dbg[:, :Wg])
                continue
            # conc column-stacked: subtile A at cols 0:512, B at 512:1024
            conc_ps = ccps.tile([TS, 1024], f32, tag="conc_ps", name="conc_ps")
            for h in range(b):
                lo = h * NT
                nc.tensor.matmul(conc_ps[:, lo:lo + NT], cmat_r,
                                 bs[:, lo:lo + NT], start=True, stop=True)

            if STAGE < 6:
                dbg = opool.tile([TS, 1024], f32, tag="dbg", name="dbg")
                V.tensor_copy(dbg[:, :Wg], conc_ps[:, :Wg])
                nc.sync.dma_start(out=sig[:, j0 * NT:j0 * NT + Wg], in_=dbg[:, :Wg])
                continue
            th_t = epool.tile([TS, 1024], f32, tag="th_t", name="th_t")
            nc.scalar.activation(
                th_t[:, :Wg], conc_ps[:, :Wg], AF.Tanh,
                bias=b_vh[0:TS, 0:1], scale=float(-TR * R1CA / 2.0))
            rt = epool.tile([TS, 1024], f32, tag="rt", name="rt")
            V.reciprocal_approx_fast(rt[:, :Wg], th_t[:, :Wg])
            out_t = opool.tile([TS, 1024], f32, tag="out_t", name="out_t")
            G.tensor_scalar(out_t[:, :Wg], rt[:, :Wg], float(K2p), float(K1p),
                            op0=ALU.mult, op1=ALU.add)
            nc.sync.dma_start(out=sig[:, j0 * NT:j0 * NT + Wg],
                              in_=out_t[:, :Wg])

    nc.compile()
    return nc


def _host_prep(sample_time: np.ndarray, Cp: np.ndarray):
    t_end = float(np.asarray(sample_time)[-1])
    Lf = int(round(t_end / STEP)) + 1
    t_samp = np.arange(Lf, dtype=np.float32) * np.float32(STEP)
    aifci = np.interp(
        t_samp.astype(np.float64),
        np.asarray(sample_time, np.float64),
        np.asarray(Cp, np.float64),
    ).astype(np.float32)
    aif = np.concatenate([np.zeros(DELAY, np.float32), aifci[:-DELAY]])
    idx = np.searchsorted(t_samp, np.asarray(sample_time, np.float32), side="left")
    idx = np.minimum(idx, Lf - 1)
    A = np.zeros((TS, Lf), np.float64)
    for k in range(TS):
        i = int(idx[k])
        A[k, : i + 1] = aif[i::-1]

    sj = np.concatenate([[0.0], np.geomspace(0.15, 588.0, KB - 1)])
    th_grid = np.geomspace(0.012, 70.0, 6000)
    E = np.exp(-STEP * np.outer(th_grid, np.arange(Lf)))
    F = E @ A.T
    B = np.exp(-STEP * np.outer(th_grid, sj))
    lam = 1e-9 * np.linalg.norm(B, 2) ** 2
    C = np.linalg.solve(B.T @ B + lam * np.eye(KB), B.T @ F).T   # [TS, KB]

    cmat = np.ascontiguousarray(C.T.astype(np.float32))          # [KB, TS]
    oblk = np.zeros((2, 128), np.float32)
    oblk[0, 0:64] = 1.0
    oblk[1, 64:128] = 1.0
    svec = (-STEP * sj[(np.arange(128) % KB)]).astype(np.float32).reshape(128, 1)
    return cmat, oblk, svec


def kernel(param: np.ndarray, sample_time: np.ndarray, Cp: np.ndarray) -> np.ndarray:
    from concourse.bass_utils import run_bass_kernel_spmd

    if "nc" not in _CACHE:
        _CACHE["nc"] = _build_bass()
    nc = _CACHE["nc"]

    cmat, oblk, svec = _host_prep(sample_time, Cp)
    pflat = np.ascontiguousarray(np.asarray(param, np.float32).reshape(4, NPIX))
    in_maps = []
    for c in range(NCORES):
        in_maps.append({
            "pmap": np.ascontiguousarray(pflat[:, c * SHARD:(c + 1) * SHARD]),
            "cmat": cmat, "oblk": oblk, "svec": svec,
        })
    ncr = int(os.environ.get("DCE_CORES", str(NCORES)))
    res = run_bass_kernel_spmd(
        nc, in_maps[:ncr], core_ids=list(range(ncr)),
        trace=bool(int(os.environ.get("DCE_TRACE", "0"))),
    )
    if res.exec_time_ns is not None:
        _CACHE["exec_time_ns"] = res.exec_time_ns
    outs = [r["sig"] for r in res.results]
    while len(outs) < NCORES:
        outs.append(np.zeros((TS, SHARD), np.float32))
    out = np.concatenate(outs, axis=1)
    return out.reshape(TS, 1, H, W)
